# revision 28
# baseline (speedup 1.0000x reference)
"""Trainium2 Bass kernel for single-token (decode) multi-head attention.

Problem: q [8,32,1,128], k/v [8,32,4096,128], mask [8,1,1,4096] (fp32)
  out = softmax(q*scale @ k^T + mask) @ v          -> [8,32,1,128]

Sharding: batch across the 8 NeuronCores (B=8 -> 1 batch per core, all 32
heads on-core; no cross-core communication).

The kernel is HBM-bandwidth-bound (~358 GB/s per core), so traffic is
compressed: K is int8-quantized per (head, h-column) with the scale folded
into q on the host (scores = sum_h k_int8[kv,h] * (q[h]*SCALE*s[h]) is
exact modulo the int8 rounding of K), and V is fp16. End-to-end rel err
~9e-3 vs the 2e-2 gate.

Engine split per head-group (software-pipelined one group so ACT's
upconvert of group g+1 is issued before exp of group g; group sizes are
small at the start to shorten the pipeline fill and small at the end to
shorten the drain):
  - DMA (both tensors on the sync HWDGE ring so the scalar engine's
    program is pure ACT compute): K int8, V fp16 +ones-column.
  - ACT: upconvert K int8 -> fp16 (func=Copy), exp, psum->SBUF stashes.
  - DVE: products k*q' via tensor_tensor with a stride-0 broadcast AP
    for q' written IN-PLACE over the upconverted k, in-place pairwise
    fold adds over h (tensor_tensor runs ~2x the rate of tensor_reduce),
    short segmented tensor_reduce, mask add; one batched reciprocal +
    one broadcast multiply at the end normalize all 32 heads at once.
  - PE: AV matmuls, p_e column as the 1-wide stationary operand:
    psum[1,129] += p_e[:,i].T @ v'[:, i-block]; the ones column makes
    psum[0,128] the softmax denominator.

kv row index kv = p*J + j (p = partition, j = row-in-partition).
"""

import os

import numpy as np

import concourse.mybir as mybir
import concourse.tile as tile
from concourse import bacc
from concourse.bass_utils import run_bass_kernel_spmd

B, N, T, H, KV = 8, 32, 1, 128, 4096
SCALE = float(H) ** -0.5
P = 128          # partitions
J = KV // P      # 32 kv rows per partition
HV = H + 1       # V width incl. ones column
GROUPS = [1, 1, 2, 4, 4, 4, 4, 4, 4, 2, 2]   # head-group sizes, sum = 32
F32 = mybir.dt.float32
F16 = mybir.dt.float16
I8 = mybir.dt.int8

_NC_CACHE = None
LAST_RESULT = None  # BassKernelResults of the most recent run (for test harness)


def _build():
    nc = bacc.Bacc()
    q_d = nc.dram_tensor("qb", [P, N * H], F16, kind="ExternalInput")
    k_d = nc.dram_tensor("kt", [P, N * J * H], I8, kind="ExternalInput")
    v_d = nc.dram_tensor("vt", [P, N * J * HV], F16, kind="ExternalInput")
    m_d = nc.dram_tensor("maskr", [P, J], F32, kind="ExternalInput")
    o_d = nc.dram_tensor("out", [1, N * H], F32, kind="ExternalOutput")

    with tile.TileContext(nc) as tc:
        with (
            tc.tile_pool(name="const", bufs=1) as const,
            tc.tile_pool(name="k8", bufs=2) as k8p,
            tc.tile_pool(name="kb", bufs=2) as kbp,
            tc.tile_pool(name="vp", bufs=8) as vp,
            tc.tile_pool(name="pr16", bufs=2) as srp,
            tc.tile_pool(name="praw", bufs=2) as prp,
            tc.tile_pool(name="pexp", bufs=2) as pep,
            tc.tile_pool(name="po", bufs=8, space="PSUM") as pop,
        ):
            qb = const.tile([P, N * H], F16)
            msk = const.tile([P, J], F32)
            stash = const.tile([1, N * HV], F16)   # per-head [num(128)|den]
            out_row = const.tile([1, N * H], F32)
            recip = const.tile([1, N], F32)

            def compute(n0, g, kb, v_tiles):
                """products -> fold-reduce -> mask -> exp -> AV -> stash for
                heads n0..n0+g-1 (runs one group behind DMA + upconvert)."""
                in1 = (
                    qb[:, n0 * H:(n0 + g) * H]
                    .rearrange("p (n h) -> p n h", n=g)
                    .unsqueeze(2)
                    .to_broadcast((P, g, J, H))
                )
                prod3 = kb[:].rearrange("p (n j h) -> p n j h", n=g, j=J)
                nc.vector.tensor_tensor(
                    out=prod3, in0=prod3, in1=in1, op=mybir.AluOpType.mult
                )

                w = H
                while w > 8:
                    w //= 2
                    nc.vector.tensor_tensor(
                        out=prod3[:, :, :, 0:w],
                        in0=prod3[:, :, :, 0:w],
                        in1=prod3[:, :, :, w:2 * w],
                        op=mybir.AluOpType.add,
                    )
                pr16 = srp.tile([P, g * J], F16)
                with nc.allow_low_precision(
                    reason="DVE reduces fp32 internally; fp16 out is plenty"
                ):
                    nc.vector.tensor_reduce(
                        out=pr16[:],
                        in_=prod3[:, :, :, 0:8],
                        axis=mybir.AxisListType.X,
                        op=mybir.AluOpType.add,
                    )

                p_raw = prp.tile([P, g * J], F32)
                nc.vector.tensor_tensor(
                    out=p_raw[:].rearrange("p (n j) -> p n j", n=g),
                    in0=pr16[:].rearrange("p (n j) -> p n j", n=g),
                    in1=msk[:].unsqueeze(1).to_broadcast((P, g, J)),
                    op=mybir.AluOpType.add,
                )

                p_e = pep.tile([P, g * J], F16)
                nc.scalar.activation(
                    out=p_e[:],
                    in_=p_raw[:],
                    func=mybir.ActivationFunctionType.Exp,
                )

                for np_ in range(g):
                    n = n0 + np_
                    v_sb = v_tiles[np_]
                    po = pop.tile([1, HV], F32)
                    for j in range(J):
                        nc.tensor.matmul(
                            po[:],
                            lhsT=p_e[:, np_ * J + j:np_ * J + j + 1],
                            rhs=v_sb[:, j * HV:(j + 1) * HV],
                            start=(j == 0),
                            stop=(j == J - 1),
                        )
                    # stash [num | den] to SBUF on the (otherwise idle) ACT
                    nc.scalar.activation(
                        out=stash[0:1, n * HV:(n + 1) * HV],
                        in_=po[0:1, :],
                        func=mybir.ActivationFunctionType.Copy,
                    )

            pending = None
            n0 = 0
            for g in GROUPS:
                k_sb = k8p.tile([P, g * J * H], I8)
                nc.sync.dma_start(
                    out=k_sb[:], in_=k_d[:, n0 * J * H:(n0 + g) * J * H]
                )
                if n0 == 0:
                    # qb/mask ride the scalar ring (its only DMAs) so the
                    # first products are not blocked behind K0/V0.
                    nc.scalar.dma_start(out=qb[:], in_=q_d[:])
                    nc.scalar.dma_start(out=msk[:], in_=m_d[:])
                # per-head V tiles on the SWDGE (gpsimd) ring: V buffer
                # waits can never block the K transfers on the sync ring
                v_tiles = []
                for np_ in range(g):
                    vh = vp.tile([P, J * HV], F16)
                    nc.gpsimd.dma_start(
                        out=vh[:],
                        in_=v_d[:, (n0 + np_) * J * HV:(n0 + np_ + 1) * J * HV],
                    )
                    v_tiles.append(vh)

                # upconvert K int8 -> fp16 on ACT (integer values, exact)
                kb = kbp.tile([P, g * J * H], F16)
                nc.scalar.activation(
                    out=kb[:], in_=k_sb[:],
                    func=mybir.ActivationFunctionType.Copy,
                )

                if pending is not None:
                    compute(*pending)
                pending = (n0, g, kb, v_tiles)
                n0 += g
            compute(*pending)

            # batched normalize: recip of all denominators, one broadcast mult
            sv = stash[:].rearrange("p (n c) -> p n c", n=N)
            nc.vector.reciprocal(out=recip[0:1, 0:N], in_=sv[:, :, H:H + 1])
            nc.vector.tensor_tensor(
                out=out_row[:].rearrange("p (n h) -> p n h", n=N),
                in0=sv[:, :, 0:H],
                in1=recip[0:1, 0:N].unsqueeze(2).to_broadcast((1, N, H)),
                op=mybir.AluOpType.mult,
            )

            nc.sync.dma_start(out=o_d[:], in_=out_row[:])
    nc.finalize()
    return nc


def kernel(q, k, v, mask):
    global _NC_CACHE, LAST_RESULT
    q = np.asarray(q, dtype=np.float32)
    k = np.asarray(k, dtype=np.float32)
    v = np.asarray(v, dtype=np.float32)
    mask = np.asarray(mask, dtype=np.float32)

    if _NC_CACHE is None:
        _NC_CACHE = _build()
    nc = _NC_CACHE

    # host-side restaging:
    #  - K: int8 per-(head, h-column) scales, folded into q'
    #  - layout [N, KV, H] -> [N, P, J, H] -> [P, N, J, H]  (kv = p*J + j)
    s_col = np.abs(k).max(axis=2, keepdims=True) / 127.0          # [B,N,1,H]
    k8 = np.clip(np.round(k / s_col), -127, 127).astype(np.int8)
    kt = k8.reshape(B, N, P, J, H).transpose(0, 2, 1, 3, 4)
    kt = np.ascontiguousarray(kt).reshape(B, P, N * J * H)
    vt = v.reshape(B, N, P, J, H).transpose(0, 2, 1, 3, 4)
    ones = np.ones((B, P, N, J, 1), dtype=np.float32)
    vt = np.concatenate([vt, ones], axis=-1)
    vt = np.ascontiguousarray(vt).astype(np.float16).reshape(B, P, N * J * HV)
    qs = (q[:, :, 0, :] * SCALE * s_col[:, :, 0, :]).astype(np.float16)
    qs = qs.reshape(B, 1, N * H)
    qb = np.broadcast_to(qs, (B, P, N * H))

    in_maps = []
    for b in range(B):
        in_maps.append({
            "qb": np.ascontiguousarray(qb[b]),
            "kt": kt[b],
            "vt": vt[b],
            "maskr": np.ascontiguousarray(mask[b, 0, 0, :].reshape(P, J)),
        })

    res = run_bass_kernel_spmd(
        nc,
        in_maps,
        core_ids=list(range(B)),
        trace=bool(int(os.environ.get("KERNEL_TRACE", "0"))),
    )
    LAST_RESULT = res
    out = np.stack([r["out"].reshape(N, H) for r in res.results])
    return out[:, :, None, :].astype(np.float32)


# revision 29
# speedup vs baseline: 1.0429x; 1.0429x over previous
"""Trainium2 Bass kernel for single-token (decode) multi-head attention.

Problem: q [8,32,1,128], k/v [8,32,4096,128], mask [8,1,1,4096] (fp32)
  out = softmax(q*scale @ k^T + mask) @ v          -> [8,32,1,128]

Sharding: batch across the 8 NeuronCores (B=8 -> 1 batch per core, all 32
heads on-core; no cross-core communication).

The kernel is HBM-bandwidth-bound (~358 GB/s per core), so traffic is
compressed: K is int8-quantized per (head, h-column) with the scale folded
into q on the host (scores = sum_h k_int8[kv,h] * (q[h]*SCALE*s[h]) is
exact modulo the int8 rounding of K), and V is fp16. End-to-end rel err
~9e-3 vs the 2e-2 gate.

Engine split per head-group (software-pipelined one group so ACT's
upconvert of group g+1 is issued before exp of group g; group sizes are
small at the start to shorten the pipeline fill and small at the end to
shorten the drain):
  - DMA (both tensors on the sync HWDGE ring so the scalar engine's
    program is pure ACT compute): K int8, V fp16 +ones-column.
  - ACT: upconvert K int8 -> fp16 (func=Copy), exp, psum->SBUF stashes.
  - DVE: products k*q' via tensor_tensor with a stride-0 broadcast AP
    for q' written IN-PLACE over the upconverted k, in-place pairwise
    fold adds over h (tensor_tensor runs ~2x the rate of tensor_reduce),
    short segmented tensor_reduce, mask add; one batched reciprocal +
    one broadcast multiply at the end normalize all 32 heads at once.
  - PE: AV matmuls, p_e column as the 1-wide stationary operand:
    psum[1,129] += p_e[:,i].T @ v'[:, i-block]; the ones column makes
    psum[0,128] the softmax denominator.

kv row index kv = p*J + j (p = partition, j = row-in-partition).
"""

import os

import numpy as np

import concourse.mybir as mybir
import concourse.tile as tile
from concourse import bacc
from concourse.bass_utils import run_bass_kernel_spmd

B, N, T, H, KV = 8, 32, 1, 128, 4096
SCALE = float(H) ** -0.5
P = 128          # partitions
J = KV // P      # 32 kv rows per partition
HV = H + 1       # V width incl. ones column
GROUPS = [1, 1, 2, 4, 4, 4, 4, 4, 4, 2, 2]   # head-group sizes, sum = 32
F32 = mybir.dt.float32
F16 = mybir.dt.float16
I8 = mybir.dt.int8

_NC_CACHE = None
LAST_RESULT = None  # BassKernelResults of the most recent run (for test harness)


def _build():
    nc = bacc.Bacc()
    q_d = nc.dram_tensor("qb", [P, N * H], F16, kind="ExternalInput")
    k_d = nc.dram_tensor("kt", [P, N * J * H], I8, kind="ExternalInput")
    v_d = nc.dram_tensor("vt", [P, N * J * HV], F16, kind="ExternalInput")
    m_d = nc.dram_tensor("maskr", [P, J], F32, kind="ExternalInput")
    o_d = nc.dram_tensor("out", [1, N * H], F32, kind="ExternalOutput")

    with tile.TileContext(nc) as tc:
        with (
            tc.tile_pool(name="const", bufs=1) as const,
            tc.tile_pool(name="k8", bufs=2) as k8p,
            tc.tile_pool(name="kb", bufs=2) as kbp,
            tc.tile_pool(name="vp", bufs=8) as vp,
            tc.tile_pool(name="pr16", bufs=2) as srp,
            tc.tile_pool(name="praw", bufs=2) as prp,
            tc.tile_pool(name="pexp", bufs=2) as pep,
            tc.tile_pool(name="po", bufs=8, space="PSUM") as pop,
        ):
            qb = const.tile([P, N * H], F16)
            msk = const.tile([P, J], F32)
            stash = const.tile([1, N * HV], F16)   # per-head [num(128)|den]
            out_row = const.tile([1, N * H], F32)
            recip = const.tile([1, N], F32)

            def compute(n0, g, kb, v_tiles):
                """products -> fold-reduce -> mask -> exp -> AV -> stash for
                heads n0..n0+g-1 (runs one group behind DMA + upconvert)."""
                in1 = (
                    qb[:, n0 * H:(n0 + g) * H]
                    .rearrange("p (n h) -> p n h", n=g)
                    .unsqueeze(2)
                    .to_broadcast((P, g, J, H))
                )
                prod3 = kb[:].rearrange("p (n j h) -> p n j h", n=g, j=J)
                nc.vector.tensor_tensor(
                    out=prod3, in0=prod3, in1=in1, op=mybir.AluOpType.mult
                )

                w = H
                while w > 8:
                    w //= 2
                    nc.vector.tensor_tensor(
                        out=prod3[:, :, :, 0:w],
                        in0=prod3[:, :, :, 0:w],
                        in1=prod3[:, :, :, w:2 * w],
                        op=mybir.AluOpType.add,
                    )
                pr16 = srp.tile([P, g * J], F16)
                with nc.allow_low_precision(
                    reason="DVE reduces fp32 internally; fp16 out is plenty"
                ):
                    nc.vector.tensor_reduce(
                        out=pr16[:],
                        in_=prod3[:, :, :, 0:8],
                        axis=mybir.AxisListType.X,
                        op=mybir.AluOpType.add,
                    )

                p_raw = prp.tile([P, g * J], F32)
                nc.vector.tensor_tensor(
                    out=p_raw[:].rearrange("p (n j) -> p n j", n=g),
                    in0=pr16[:].rearrange("p (n j) -> p n j", n=g),
                    in1=msk[:].unsqueeze(1).to_broadcast((P, g, J)),
                    op=mybir.AluOpType.add,
                )

                p_e = pep.tile([P, g * J], F16)
                nc.scalar.activation(
                    out=p_e[:],
                    in_=p_raw[:],
                    func=mybir.ActivationFunctionType.Exp,
                )

                for np_ in range(g):
                    n = n0 + np_
                    v_sb = v_tiles[np_]
                    po = pop.tile([1, HV], F32)
                    for j in range(J):
                        nc.tensor.matmul(
                            po[:],
                            lhsT=p_e[:, np_ * J + j:np_ * J + j + 1],
                            rhs=v_sb[:, j * HV:(j + 1) * HV],
                            start=(j == 0),
                            stop=(j == J - 1),
                        )
                    # stash [num | den] to SBUF on the (otherwise idle) ACT
                    nc.scalar.activation(
                        out=stash[0:1, n * HV:(n + 1) * HV],
                        in_=po[0:1, :],
                        func=mybir.ActivationFunctionType.Copy,
                    )

            pending = None
            n0 = 0
            for g in GROUPS:
                k_sb = k8p.tile([P, g * J * H], I8)
                nc.sync.dma_start(
                    out=k_sb[:], in_=k_d[:, n0 * J * H:(n0 + g) * J * H]
                )
                if n0 == 0:
                    # qb/mask ride the scalar ring (its only DMAs) so the
                    # first products are not blocked behind K0/V0.
                    nc.scalar.dma_start(out=qb[:], in_=q_d[:])
                    nc.scalar.dma_start(out=msk[:], in_=m_d[:])
                # per-head V tiles: buffers free at head granularity so a
                # pending V transfer never blocks K transfers behind it in
                # the ring for long
                v_tiles = []
                for np_ in range(g):
                    vh = vp.tile([P, J * HV], F16)
                    nc.sync.dma_start(
                        out=vh[:],
                        in_=v_d[:, (n0 + np_) * J * HV:(n0 + np_ + 1) * J * HV],
                    )
                    v_tiles.append(vh)

                # upconvert K int8 -> fp16 on ACT (integer values, exact)
                kb = kbp.tile([P, g * J * H], F16)
                nc.scalar.activation(
                    out=kb[:], in_=k_sb[:],
                    func=mybir.ActivationFunctionType.Copy,
                )

                if pending is not None:
                    compute(*pending)
                pending = (n0, g, kb, v_tiles)
                n0 += g
            compute(*pending)

            # batched normalize: recip of all denominators, one broadcast mult
            sv = stash[:].rearrange("p (n c) -> p n c", n=N)
            nc.vector.reciprocal(out=recip[0:1, 0:N], in_=sv[:, :, H:H + 1])
            nc.vector.tensor_tensor(
                out=out_row[:].rearrange("p (n h) -> p n h", n=N),
                in0=sv[:, :, 0:H],
                in1=recip[0:1, 0:N].unsqueeze(2).to_broadcast((1, N, H)),
                op=mybir.AluOpType.mult,
            )

            nc.sync.dma_start(out=o_d[:], in_=out_row[:])
    nc.finalize()
    return nc


def kernel(q, k, v, mask):
    global _NC_CACHE, LAST_RESULT
    q = np.asarray(q, dtype=np.float32)
    k = np.asarray(k, dtype=np.float32)
    v = np.asarray(v, dtype=np.float32)
    mask = np.asarray(mask, dtype=np.float32)

    if _NC_CACHE is None:
        _NC_CACHE = _build()
    nc = _NC_CACHE

    # host-side restaging:
    #  - K: int8 per-(head, h-column) scales, folded into q'
    #  - layout [N, KV, H] -> [N, P, J, H] -> [P, N, J, H]  (kv = p*J + j)
    s_col = np.abs(k).max(axis=2, keepdims=True) / 127.0          # [B,N,1,H]
    k8 = np.clip(np.round(k / s_col), -127, 127).astype(np.int8)
    kt = k8.reshape(B, N, P, J, H).transpose(0, 2, 1, 3, 4)
    kt = np.ascontiguousarray(kt).reshape(B, P, N * J * H)
    vt = v.reshape(B, N, P, J, H).transpose(0, 2, 1, 3, 4)
    ones = np.ones((B, P, N, J, 1), dtype=np.float32)
    vt = np.concatenate([vt, ones], axis=-1)
    vt = np.ascontiguousarray(vt).astype(np.float16).reshape(B, P, N * J * HV)
    qs = (q[:, :, 0, :] * SCALE * s_col[:, :, 0, :]).astype(np.float16)
    qs = qs.reshape(B, 1, N * H)
    qb = np.broadcast_to(qs, (B, P, N * H))

    in_maps = []
    for b in range(B):
        in_maps.append({
            "qb": np.ascontiguousarray(qb[b]),
            "kt": kt[b],
            "vt": vt[b],
            "maskr": np.ascontiguousarray(mask[b, 0, 0, :].reshape(P, J)),
        })

    res = run_bass_kernel_spmd(
        nc,
        in_maps,
        core_ids=list(range(B)),
        trace=bool(int(os.environ.get("KERNEL_TRACE", "0"))),
    )
    LAST_RESULT = res
    out = np.stack([r["out"].reshape(N, H) for r in res.results])
    return out[:, :, None, :].astype(np.float32)
